# revision 2
# baseline (speedup 1.0000x reference)
"""Trainium2 Bass kernel for the MHC layer (nn_MHCLayer_20555713478899).

Reference computation (per batch row b of x[B=8192, n=4, C=4096] f32):
    hpre = sigmoid(H_pre)                     # [4]
    x_agg[b, c]   = sum_n hpre[n] * x[b, n, c]
    x_agg_bf      = bf16_roundtrip(x_agg)
    rms[b]        = sqrt(mean_c(x_agg_bf^2) + 1e-6)
    y_norm[b, c]  = x_agg_bf / rms * rmsnorm_weight[c]
    P             = sinkhorn3(exp(H_res))     # [4, 4]  (tiny, host-computed)
    hpost = 2*sigmoid(H_post)                 # [4]
    out[b, i, c]  = sum_j P[i, j] * x[b, j, c] + hpost[i] * y_norm[b, c]

Strategy: data-parallel shard of B across 8 NeuronCores (1024 rows each).
On-chip, batches are processed in supertiles of 128 rows = 4 subtiles of 32
rows.  A 32-row subtile of x loads as a contiguous [128, 4096] SBUF tile whose
partition index is (bg*4 + n) — so all the n-mixing becomes 128x128 matmuls
with small block-structured host-built matrices:
  mm_agg : lhsT wpre_s[(bg,n), 32s+bg] = hpre[n]     -> x_agg rows (32s+bg)
  mm_mix : lhsT blockP[(bg,j), (bg,i)] = P[i,j]      -> mixed rows  (bg,i)
  mm_post: lhsT bpost_s[r, (bg,i)] = hpost[i]*d(r=32s+bg), rhs y_norm
           accumulated into the same PSUM as mm_mix -> + hpost[i]*y_norm
The RMS-norm path runs on ACT/DVE.  All matmul operands are bf16 (PSUM
accumulation is f32); loads cast f32->bf16 in-flight via SWDGE DMA.
"""

import os

import numpy as np
import ml_dtypes

import concourse.bass as bass
import concourse.tile as tile
from concourse import bacc, mybir
from concourse.bass_utils import run_bass_kernel_spmd

B, N, C = 8192, 4, 4096
NCORES = 8
BLOC = B // NCORES          # 1024 batch rows per core
SUB = 32                    # batch rows per subtile (SUB*N = 128 partitions)
NSUB = 4                    # subtiles per supertile
ST = SUB * NSUB             # 128 batch rows per supertile
EPS = 1e-6
SINKHORN_ITERS = 3

F32 = mybir.dt.float32
BF16 = mybir.dt.bfloat16
BF16_NP = ml_dtypes.bfloat16

_PROGRAM = None
LAST_RESULTS = None         # BassKernelResults of the last run (for profiling)


def _build_program(bloc=BLOC):
    nc = bacc.Bacc("TRN2", target_bir_lowering=False)

    x_d = nc.dram_tensor("x", [bloc, N, C], F32, kind="ExternalInput")
    wrep_d = nc.dram_tensor("wrep", [128, C], BF16, kind="ExternalInput")
    blockp_d = nc.dram_tensor("blockp", [128, 128], BF16, kind="ExternalInput")
    wpre_d = nc.dram_tensor("wpre", [128, NSUB, 128], BF16, kind="ExternalInput")
    bpost_d = nc.dram_tensor("bpost", [128, NSUB, 128], BF16, kind="ExternalInput")
    out_d = nc.dram_tensor("out", [bloc, N, C], F32, kind="ExternalOutput")

    n_st = bloc // ST
    AluOp = mybir.AluOpType
    Act = mybir.ActivationFunctionType

    with tile.TileContext(nc) as tc:
        with (
            tc.tile_pool(name="consts", bufs=1) as consts,
            tc.tile_pool(name="xbf", bufs=8) as xbf_pool,
            tc.tile_pool(name="norm", bufs=2) as norm_pool,
            tc.tile_pool(name="yn", bufs=2) as yn_pool,
            tc.tile_pool(name="scr", bufs=2) as scr_pool,
            tc.tile_pool(name="small", bufs=4) as small_pool,
            tc.tile_pool(name="osb", bufs=3) as out_pool,
            tc.tile_pool(name="aggps", bufs=2, space=bass.MemorySpace.PSUM) as agg_pool,
            tc.tile_pool(name="mixps", bufs=2, space=bass.MemorySpace.PSUM) as mix_pool,
        ):
            wrep_t = consts.tile([128, C], BF16, tag="wrep")
            nc.sync.dma_start(wrep_t[:], wrep_d[:])
            blockp_t = consts.tile([128, 128], BF16, tag="blockp")
            nc.sync.dma_start(blockp_t[:], blockp_d[:])
            wpre_t = consts.tile([128, NSUB, 128], BF16, tag="wpre")
            nc.sync.dma_start(wpre_t[:], wpre_d[:])
            bpost_t = consts.tile([128, NSUB, 128], BF16, tag="bpost")
            nc.sync.dma_start(bpost_t[:], bpost_d[:])
            eps_t = consts.tile([128, 1], F32, tag="eps")
            nc.vector.memset(eps_t[:], EPS)

            for t in range(n_st):
                b0 = t * ST

                # ---- load subtiles, casting f32 -> bf16 in the DMA ----
                xs = []
                for s in range(NSUB):
                    xt = xbf_pool.tile([128, C], BF16, tag="xbf")
                    nc.gpsimd.dma_start(
                        out=xt[:], in_=x_d[b0 + SUB * s : b0 + SUB * (s + 1)]
                    )
                    xs.append(xt)

                # ---- x_agg via PE; evacuate+cast to bf16 via ACT ----
                xagg = norm_pool.tile([128, C], BF16, tag="xagg")
                for qp in range(2):  # pairs of 1024-col quarters
                    aggts = []
                    for j in range(2):
                        at = agg_pool.tile([128, 1024], F32, tag="agg")
                        aggts.append(at)
                    for s in range(NSUB):
                        for j, at in enumerate(aggts):
                            q = qp * 2 + j
                            for c2 in range(2):
                                lo = q * 1024 + c2 * 512
                                nc.tensor.matmul(
                                    at[:, c2 * 512 : (c2 + 1) * 512],
                                    wpre_t[:, s, :],
                                    xs[s][:, lo : lo + 512],
                                    start=(s == 0),
                                    stop=(s == NSUB - 1),
                                )
                    for j, at in enumerate(aggts):
                        q = qp * 2 + j
                        nc.scalar.copy(xagg[:, q * 1024 : (q + 1) * 1024], at[:])

                # ---- rms over C (matches ref: square bf16 values in f32) ----
                # ACT Square with accum_out fuses square + free-dim reduce.
                # (tensor_tensor_reduce wedges the device on this runtime.)
                sq4 = small_pool.tile([128, 4], F32, tag="sq4")
                scratch = scr_pool.tile([128, 1024], F32, tag="scr")
                for q in range(4):
                    xa_q = xagg[:, q * 1024 : (q + 1) * 1024]
                    nc.scalar.activation(
                        scratch[:],
                        xa_q,
                        Act.Square,
                        accum_out=sq4[:, q : q + 1],
                    )
                sumsq = small_pool.tile([128, 1], F32, tag="sumsq")
                nc.vector.tensor_reduce(
                    sumsq[:], sq4[:], mybir.AxisListType.X, AluOp.add
                )
                rmsv = small_pool.tile([128, 1], F32, tag="rmsv")
                nc.scalar.activation(
                    rmsv[:], sumsq[:], Act.Sqrt, bias=eps_t[:], scale=1.0 / C
                )
                invr = small_pool.tile([128, 1], F32, tag="invr")
                nc.vector.reciprocal(invr[:], rmsv[:])

                # ---- y_norm = x_agg_bf * invr * w  (bf16 for the PE) ----
                yn = yn_pool.tile([128, C], BF16, tag="yn")
                nc.vector.tensor_scalar_mul(yn[:], xagg[:], invr[:])
                nc.vector.tensor_mul(yn[:], yn[:], wrep_t[:])

                # ---- mix + post-add on PE; evacuate; store ----
                for s in range(NSUB):
                    osb = out_pool.tile([128, C], F32, tag="osb")
                    for qp in range(2):
                        mixts = []
                        for j in range(2):
                            mt = mix_pool.tile([128, 1024], F32, tag="mix")
                            mixts.append(mt)
                        for j, mt in enumerate(mixts):
                            q = qp * 2 + j
                            for c2 in range(2):
                                lo = q * 1024 + c2 * 512
                                nc.tensor.matmul(
                                    mt[:, c2 * 512 : (c2 + 1) * 512],
                                    blockp_t[:],
                                    xs[s][:, lo : lo + 512],
                                    start=True,
                                    stop=False,
                                )
                        for j, mt in enumerate(mixts):
                            q = qp * 2 + j
                            for c2 in range(2):
                                lo = q * 1024 + c2 * 512
                                nc.tensor.matmul(
                                    mt[:, c2 * 512 : (c2 + 1) * 512],
                                    bpost_t[:, s, :],
                                    yn[:, lo : lo + 512],
                                    start=False,
                                    stop=True,
                                )
                        for j, mt in enumerate(mixts):
                            q = qp * 2 + j
                            dst = osb[:, q * 1024 : (q + 1) * 1024]
                            if q % 2 == 0:
                                nc.vector.tensor_copy(dst, mt[:])
                            else:
                                nc.scalar.copy(dst, mt[:])
                    nc.sync.dma_start(
                        out=out_d[b0 + SUB * s : b0 + SUB * (s + 1)], in_=osb[:]
                    )

    nc.compile()
    return nc


def _sigmoid_f32(x):
    x = np.asarray(x, np.float32)
    return (1.0 / (1.0 + np.exp(-x.astype(np.float64)))).astype(np.float32)


def _host_matrices(rmsnorm_weight, H_pre, H_post, H_res):
    f32 = np.float32
    hpre = _sigmoid_f32(H_pre)                        # [4]
    hpost = (2.0 * _sigmoid_f32(H_post)).astype(f32)  # [4]
    P = np.exp(np.asarray(H_res, f32))
    for _ in range(SINKHORN_ITERS):
        P = P / (P.sum(axis=-1, keepdims=True) + f32(EPS))
        P = P / (P.sum(axis=-2, keepdims=True) + f32(EPS))
    P = P.astype(f32)

    blockp = np.zeros((128, 128), f32)
    for bg in range(SUB):
        # out[(bg,i), c] = sum_j blockp[(bg,j), (bg,i)] * x[(bg,j), c]
        blockp[4 * bg : 4 * bg + 4, 4 * bg : 4 * bg + 4] = P.T

    wpre = np.zeros((128, NSUB, 128), f32)
    bpost = np.zeros((128, NSUB, 128), f32)
    for s in range(NSUB):
        for bg in range(SUB):
            for n in range(4):
                wpre[4 * bg + n, s, SUB * s + bg] = hpre[n]
            for i in range(4):
                bpost[SUB * s + bg, s, 4 * bg + i] = hpost[i]

    wrep = np.broadcast_to(
        np.asarray(rmsnorm_weight, f32)[None, :], (128, C)
    )
    return {
        "wrep": np.ascontiguousarray(wrep.astype(BF16_NP)),
        "blockp": blockp.astype(BF16_NP),
        "wpre": wpre.astype(BF16_NP),
        "bpost": bpost.astype(BF16_NP),
    }


def kernel(x, rmsnorm_weight, H_pre, H_post, H_res):
    global _PROGRAM, LAST_RESULTS
    x = np.ascontiguousarray(np.asarray(x, np.float32))
    assert x.shape == (B, N, C), x.shape

    if _PROGRAM is None:
        _PROGRAM = _build_program()
    nc = _PROGRAM

    consts = _host_matrices(rmsnorm_weight, H_pre, H_post, H_res)
    shards = np.split(x, NCORES, axis=0)
    in_maps = [{"x": np.ascontiguousarray(s), **consts} for s in shards]

    trace = bool(int(os.environ.get("MHC_TRACE", "0")))
    br = run_bass_kernel_spmd(
        nc, in_maps, core_ids=list(range(NCORES)), trace=trace
    )
    LAST_RESULTS = br
    out = np.concatenate([r["out"] for r in br.results], axis=0)
    return out


# revision 5
# speedup vs baseline: 74950.0926x; 74950.0926x over previous
"""Trainium2 Bass kernel for the MHC layer (nn_MHCLayer_20555713478899).

Reference computation (per batch row b of x[B=8192, n=4, C=4096] f32):
    hpre = sigmoid(H_pre)                     # [4]
    x_agg[b, c]   = sum_n hpre[n] * x[b, n, c]
    x_agg_bf      = bf16_roundtrip(x_agg)
    rms[b]        = sqrt(mean_c(x_agg_bf^2) + 1e-6)
    y_norm[b, c]  = x_agg_bf / rms * rmsnorm_weight[c]
    P             = sinkhorn3(exp(H_res))     # [4, 4]  (tiny, host-computed)
    hpost = 2*sigmoid(H_post)                 # [4]
    out[b, i, c]  = sum_j P[i, j] * x[b, j, c] + hpost[i] * y_norm[b, c]

Strategy: data-parallel shard of B across 8 NeuronCores (1024 rows each).
On-chip, batches are processed in supertiles of 128 rows = 4 subtiles of 32
rows.  A 32-row subtile of x loads as a contiguous [128, 4096] SBUF tile whose
partition index is (bg*4 + n) — so all the n-mixing becomes 128x128 matmuls
with small block-structured host-built matrices:
  mm_agg : lhsT wpre_s[(bg,n), 32s+bg] = hpre[n]     -> x_agg rows (32s+bg)
  mm_mix : lhsT blockP[(bg,j), (bg,i)] = P[i,j]      -> mixed rows  (bg,i)
  mm_post: lhsT bpost_s[r, (bg,i)] = hpost[i]*d(r=32s+bg), rhs y_norm
           accumulated into the same PSUM as mm_mix -> + hpost[i]*y_norm
The RMS-norm path runs on ACT/DVE.  All matmul operands are bf16 (PSUM
accumulation is f32); loads cast f32->bf16 in-flight via SWDGE DMA.
"""

import contextlib
import os

import numpy as np
import ml_dtypes

import concourse.bass as bass
import concourse.tile as tile
from concourse import bacc, mybir
from concourse.bass_utils import run_bass_kernel_spmd

B, N, C = 8192, 4, 4096
NCORES = 8
BLOC = B // NCORES          # 1024 batch rows per core
SUB = 32                    # batch rows per subtile (SUB*N = 128 partitions)
NSUB = 4                    # subtiles per supertile
ST = SUB * NSUB             # 128 batch rows per supertile
EPS = 1e-6
SINKHORN_ITERS = 3

F32 = mybir.dt.float32
BF16 = mybir.dt.bfloat16
BF16_NP = ml_dtypes.bfloat16

_PROGRAM = None
LAST_RESULTS = None         # BassKernelResults of the last run (for profiling)


def _build_program(bloc=BLOC, repeat=1):
    nc = bacc.Bacc("TRN2", target_bir_lowering=False)

    x_d = nc.dram_tensor("x", [bloc, N, C], F32, kind="ExternalInput")
    wrep_d = nc.dram_tensor("wrep", [128, C], BF16, kind="ExternalInput")
    blockp_d = nc.dram_tensor("blockp", [128, 128], BF16, kind="ExternalInput")
    wpre_d = nc.dram_tensor("wpre", [128, NSUB, 128], BF16, kind="ExternalInput")
    bpost_d = nc.dram_tensor("bpost", [128, NSUB, 128], BF16, kind="ExternalInput")
    out_d = nc.dram_tensor("out", [bloc, N, C], F32, kind="ExternalOutput")

    n_st = bloc // ST
    AluOp = mybir.AluOpType
    Act = mybir.ActivationFunctionType

    with tile.TileContext(nc) as tc:
        with (
            tc.tile_pool(name="consts", bufs=1) as consts,
            tc.tile_pool(name="xbf", bufs=8) as xbf_pool,
            tc.tile_pool(name="norm", bufs=2) as norm_pool,
            tc.tile_pool(name="yn", bufs=2) as yn_pool,
            tc.tile_pool(name="scr", bufs=2) as scr_pool,
            tc.tile_pool(name="small", bufs=4) as small_pool,
            tc.tile_pool(name="osb", bufs=3) as out_pool,
            tc.tile_pool(name="aggps", bufs=2, space=bass.MemorySpace.PSUM) as agg_pool,
            tc.tile_pool(name="mixps", bufs=2, space=bass.MemorySpace.PSUM) as mix_pool,
        ):
            wrep_t = consts.tile([128, C], BF16, tag="wrep", name="wrep_t")
            nc.sync.dma_start(wrep_t[:], wrep_d[:])
            blockp_t = consts.tile([128, 128], BF16, tag="blockp", name="blockp_t")
            nc.sync.dma_start(blockp_t[:], blockp_d[:])
            wpre_t = consts.tile([128, NSUB, 128], BF16, tag="wpre", name="wpre_t")
            nc.sync.dma_start(wpre_t[:], wpre_d[:])
            bpost_t = consts.tile([128, NSUB, 128], BF16, tag="bpost", name="bpost_t")
            nc.sync.dma_start(bpost_t[:], bpost_d[:])
            eps_t = consts.tile([128, 1], F32, tag="eps", name="eps_t")
            nc.vector.memset(eps_t[:], EPS)

            loop_cm = (
                tc.For_i(0, repeat, 1) if repeat > 1 else contextlib.nullcontext()
            )
            with loop_cm:
                for t in range(n_st):
                    b0 = t * ST

                    # ---- load subtiles, casting f32 -> bf16 in the DMA ----
                    xs = []
                    for s in range(NSUB):
                        xt = xbf_pool.tile([128, C], BF16, tag="xbf", name=f"x_{t}_{s}")
                        nc.gpsimd.dma_start(
                            out=xt[:], in_=x_d[b0 + SUB * s : b0 + SUB * (s + 1)]
                        )
                        xs.append(xt)

                    # ---- x_agg via PE; evacuate+cast to bf16 via ACT ----
                    xagg = norm_pool.tile([128, C], BF16, tag="xagg", name=f"xagg_{t}")
                    for qp in range(2):  # pairs of 1024-col quarters
                        aggts = []
                        for j in range(2):
                            at = agg_pool.tile(
                                [128, 1024], F32, tag="agg", name=f"agg_{t}_{qp}_{j}"
                            )
                            aggts.append(at)
                        for s in range(NSUB):
                            for j, at in enumerate(aggts):
                                q = qp * 2 + j
                                for c2 in range(2):
                                    lo = q * 1024 + c2 * 512
                                    nc.tensor.matmul(
                                        at[:, c2 * 512 : (c2 + 1) * 512],
                                        wpre_t[:, s, :],
                                        xs[s][:, lo : lo + 512],
                                        start=(s == 0),
                                        stop=(s == NSUB - 1),
                                    )
                        for j, at in enumerate(aggts):
                            q = qp * 2 + j
                            nc.scalar.copy(xagg[:, q * 1024 : (q + 1) * 1024], at[:])

                    # ---- rms over C (matches ref: square bf16 values in f32) ----
                    # ACT Square with accum_out fuses square + free-dim reduce.
                    # (tensor_tensor_reduce wedges the device on this runtime.)
                    sq4 = small_pool.tile([128, 4], F32, tag="sq4", name=f"sq4_{t}")
                    scratch = scr_pool.tile(
                        [128, 1024], F32, tag="scr", name=f"scr_{t}"
                    )
                    for q in range(4):
                        xa_q = xagg[:, q * 1024 : (q + 1) * 1024]
                        nc.scalar.activation(
                            scratch[:],
                            xa_q,
                            Act.Square,
                            accum_out=sq4[:, q : q + 1],
                        )
                    sumsq = small_pool.tile([128, 1], F32, tag="sumsq", name=f"ss_{t}")
                    nc.vector.tensor_reduce(
                        sumsq[:], sq4[:], mybir.AxisListType.X, AluOp.add
                    )
                    rmsv = small_pool.tile([128, 1], F32, tag="rmsv", name=f"rms_{t}")
                    nc.scalar.activation(
                        rmsv[:], sumsq[:], Act.Sqrt, bias=eps_t[:], scale=1.0 / C
                    )
                    invr = small_pool.tile([128, 1], F32, tag="invr", name=f"invr_{t}")
                    nc.vector.reciprocal(invr[:], rmsv[:])

                    # ---- y_norm = x_agg_bf * invr * w  (bf16 for the PE) ----
                    yn = yn_pool.tile([128, C], BF16, tag="yn", name=f"yn_{t}")
                    nc.vector.tensor_scalar_mul(yn[:], xagg[:], invr[:])
                    nc.vector.tensor_mul(yn[:], yn[:], wrep_t[:])

                    # ---- mix + post-add on PE; evacuate; store ----
                    for s in range(NSUB):
                        osb = out_pool.tile(
                            [128, C], F32, tag="osb", name=f"osb_{t}_{s}"
                        )
                        for qp in range(2):
                            mixts = []
                            for j in range(2):
                                mt = mix_pool.tile(
                                    [128, 1024], F32, tag="mix",
                                    name=f"mix_{t}_{s}_{qp}_{j}",
                                )
                                mixts.append(mt)
                            for j, mt in enumerate(mixts):
                                q = qp * 2 + j
                                for c2 in range(2):
                                    lo = q * 1024 + c2 * 512
                                    nc.tensor.matmul(
                                        mt[:, c2 * 512 : (c2 + 1) * 512],
                                        blockp_t[:],
                                        xs[s][:, lo : lo + 512],
                                        start=True,
                                        stop=False,
                                    )
                            for j, mt in enumerate(mixts):
                                q = qp * 2 + j
                                for c2 in range(2):
                                    lo = q * 1024 + c2 * 512
                                    nc.tensor.matmul(
                                        mt[:, c2 * 512 : (c2 + 1) * 512],
                                        bpost_t[:, s, :],
                                        yn[:, lo : lo + 512],
                                        start=False,
                                        stop=True,
                                    )
                            for j, mt in enumerate(mixts):
                                q = qp * 2 + j
                                dst = osb[:, q * 1024 : (q + 1) * 1024]
                                if q % 2 == 0:
                                    nc.vector.tensor_copy(dst, mt[:])
                                else:
                                    nc.scalar.copy(dst, mt[:])
                        nc.sync.dma_start(
                            out=out_d[b0 + SUB * s : b0 + SUB * (s + 1)], in_=osb[:]
                        )

    nc.compile()
    return nc


def _sigmoid_f32(x):
    x = np.asarray(x, np.float32)
    return (1.0 / (1.0 + np.exp(-x.astype(np.float64)))).astype(np.float32)


def _host_matrices(rmsnorm_weight, H_pre, H_post, H_res):
    f32 = np.float32
    hpre = _sigmoid_f32(H_pre)                        # [4]
    hpost = (2.0 * _sigmoid_f32(H_post)).astype(f32)  # [4]
    P = np.exp(np.asarray(H_res, f32))
    for _ in range(SINKHORN_ITERS):
        P = P / (P.sum(axis=-1, keepdims=True) + f32(EPS))
        P = P / (P.sum(axis=-2, keepdims=True) + f32(EPS))
    P = P.astype(f32)

    blockp = np.zeros((128, 128), f32)
    for bg in range(SUB):
        # out[(bg,i), c] = sum_j blockp[(bg,j), (bg,i)] * x[(bg,j), c]
        blockp[4 * bg : 4 * bg + 4, 4 * bg : 4 * bg + 4] = P.T

    wpre = np.zeros((128, NSUB, 128), f32)
    bpost = np.zeros((128, NSUB, 128), f32)
    for s in range(NSUB):
        for bg in range(SUB):
            for n in range(4):
                wpre[4 * bg + n, s, SUB * s + bg] = hpre[n]
            for i in range(4):
                bpost[SUB * s + bg, s, 4 * bg + i] = hpost[i]

    wrep = np.broadcast_to(
        np.asarray(rmsnorm_weight, f32)[None, :], (128, C)
    )
    return {
        "wrep": np.ascontiguousarray(wrep.astype(BF16_NP)),
        "blockp": blockp.astype(BF16_NP),
        "wpre": wpre.astype(BF16_NP),
        "bpost": bpost.astype(BF16_NP),
    }


def kernel(x, rmsnorm_weight, H_pre, H_post, H_res):
    global _PROGRAM, LAST_RESULTS
    x = np.ascontiguousarray(np.asarray(x, np.float32))
    assert x.shape == (B, N, C), x.shape

    if _PROGRAM is None:
        _PROGRAM = _build_program()
    nc = _PROGRAM

    consts = _host_matrices(rmsnorm_weight, H_pre, H_post, H_res)
    shards = np.split(x, NCORES, axis=0)
    in_maps = [{"x": np.ascontiguousarray(s), **consts} for s in shards]

    trace = bool(int(os.environ.get("MHC_TRACE", "0")))
    br = run_bass_kernel_spmd(
        nc, in_maps, core_ids=list(range(NCORES)), trace=trace
    )
    LAST_RESULTS = br
    out = np.concatenate([r["out"] for r in br.results], axis=0)
    return out
